# revision 18
# baseline (speedup 1.0000x reference)
"""Trainium2 Bass kernel for GQA attention (B=1,T=2048,D=3584,N=28,KH=4,H=128).

v3 sharding: heads x sequence 2D split over 8 cores.
  Pair p = cores (2p, 2p+1) owns kv head p and query heads 7p..7p+6.
  Within a pair, lane l = core % 2 owns the even (l=0) / odd (l=1)
  128-token query blocks -- the even/odd interleave is the optimal
  balanced causal split (both lanes sum to 68 key-chunk visits, and the
  SPMD union profile ceil((16-sc)/2) adds only 4 phantom chunk-columns).

Per core:
  S1: 9 projection units (7 q heads + k + v) x 28 D-chunks over my 1024
      tokens. K is roped, V transposed to natural layout; both go into a
      pairwise DRAM AllGather so each core gets K/V for all 2048 keys
      while the PE streams the Q projections (collective fully hidden).
  S2: per (head, key chunk): scoresT = kT_chunk^T qT over the union
      suffix of my query blocks >= chunk, one data-driven mask add per
      chunk (tri/full/zero content from host), exp on ACT, den/av
      accumulated on alternating PSUM banks, DVE normalize -> attnT
      (aliased onto the qT tile).
  S3: y[my tokens, :] = sum_h attnT_h^T wo_h, wo streamed per 512-col
      tile, drains alternating ACT/DVE, host sums the 4 head-group
      partials per token row.
"""

import numpy as np
import ml_dtypes
from contextlib import ExitStack

import concourse.bass as bass
import concourse.bacc as bacc
import concourse.tile as tile
from concourse import mybir
from concourse.bass_utils import run_bass_kernel_spmd

F32 = mybir.dt.float32
BF16 = mybir.dt.bfloat16

B, T, D = 1, 2048, 3584
N, KH, H = 28, 4, 128
NQ = 7                   # query heads per core
NU = NQ + 2              # + k, v units
DC = D // 128            # 28 contraction chunks
TS = 512
NB = 16                  # 128-token blocks in T
MYB = 8                  # my blocks per core
SCALE = float(H) ** -0.5
MASKVAL = -30000.0

# union suffix profile: #active 128-col blocks at key chunk sc, per stripe
W_ST = {
    0: [4, 4, 3, 3, 2, 2, 1, 1],
    1: [4, 4, 4, 4, 4, 4, 4, 4, 4, 4, 3, 3, 2, 2, 1, 1],
}

_TRACE = False           # test.py flips this to get an NTFF profile


def build_program():
    nc = bacc.Bacc(None)
    _build_body(nc)
    nc.compile()
    return nc


def _build_body(nc):
    xT_d = nc.dram_tensor("xT16", [2, 128, DC, TS], BF16, kind="ExternalInput")
    wkv_d = nc.dram_tensor("wkv16", [128, DC, 2, 128], BF16,
                           kind="ExternalInput")
    wq_d = nc.dram_tensor("wq16", [128, DC, NQ, 128], BF16,
                          kind="ExternalInput")
    bias_d = nc.dram_tensor("biasT", [128, NU], F32, kind="ExternalInput")
    cos_d = nc.dram_tensor("cosT", [128, 1024], BF16, kind="ExternalInput")
    sin_d = nc.dram_tensor("sinT", [128, 1024], BF16, kind="ExternalInput")
    ident_d = nc.dram_tensor("ident", [128, 128], F32, kind="ExternalInput")
    mask_d = nc.dram_tensor("maskC", [128, NB, 128], BF16,
                            kind="ExternalInput")
    wo_d = nc.dram_tensor("woT", [128, NQ, D], BF16, kind="ExternalInput")
    ones_d = nc.dram_tensor("ones", [128, 128], BF16, kind="ExternalInput")
    y_d = nc.dram_tensor("y", [1024, D], BF16, kind="ExternalOutput")

    with tile.TileContext(nc) as tc, ExitStack() as ctx:
        persist = ctx.enter_context(tc.tile_pool(name="persist", bufs=1))

        wkv_sb = persist.tile([128, DC, 2, 128], BF16, tag="wkv")
        wq_sb = persist.tile([128, DC, NQ, 128], BF16, tag="wq")
        x_sb = [persist.tile([128, DC, TS], BF16, tag=f"x{st}",
                             name=f"x{st}")
                for st in (0, 1)]
        # qaT[u]: S1 writes roped qT here; S2 drains overwrite it with attnT
        # (last q read of a head-stripe precedes its attnT write).
        qaT = [persist.tile([128, 1024], BF16, tag=f"qaT{u}",
                            name=f"qaT{u}")
               for u in range(NQ)]
        blob = persist.tile([128, 2048], BF16, tag="blob")   # kT|v_nat mine
        kT_full = persist.tile([128, NB, 128], BF16, tag="ktf")
        v_nat = persist.tile([128, NB, 128], BF16, tag="vnat")
        cos_sb = persist.tile([128, 1024], BF16, tag="cos")
        sin_sb = persist.tile([128, 1024], BF16, tag="sin")
        bias_sb = persist.tile([128, NU], F32, tag="bias")
        ident_sb = persist.tile([128, 128], F32, tag="ident")
        ones_sb = persist.tile([128, 128], BF16, tag="ones")
        mask_sb = persist.tile([128, NB, 128], BF16, tag="mask")

        tpool = ctx.enter_context(tc.tile_pool(name="tp", bufs=3))
        spool = ctx.enter_context(tc.tile_pool(name="sp", bufs=2))
        ppool = ctx.enter_context(tc.tile_pool(name="pp", bufs=5))
        rpool = ctx.enter_context(tc.tile_pool(name="rp", bufs=2))
        wpool = ctx.enter_context(tc.tile_pool(name="wp", bufs=3))
        ypool = ctx.enter_context(tc.tile_pool(name="yp", bufs=4))
        ps = ctx.enter_context(tc.tile_pool(name="ps", bufs=1, space="PSUM"))
        dram = ctx.enter_context(tc.tile_pool(name="dram", bufs=1,
                                              space="DRAM"))

        in_b = dram.tile([128, 2048], BF16, tag="inb")
        out_b = dram.tile([2, 128, NB, 128], BF16, tag="outb")

        # ---------------- loads (three queues, consumption order) --------
        # sync queue carries the PE-critical stream in exact consumption
        # order (KV st0 -> Q st0 grp1); scalar queue brings the later
        # wq halves + x stripe 1; gpsimd queue the small tables.
        nc.sync.dma_start(wkv_sb[:, 0:14, :, :], wkv_d[:, 0:14, :, :])
        nc.sync.dma_start(x_sb[0][:, 0:7, :], xT_d[0, :, 0:7, :])
        nc.sync.dma_start(wkv_sb[:, 14:28, :, :], wkv_d[:, 14:28, :, :])
        for g in range(1, 4):
            gs = slice(7 * g, 7 * g + 7)
            nc.sync.dma_start(x_sb[0][:, gs, :], xT_d[0, :, gs, :])
        for g in range(4):
            gs = slice(7 * g, 7 * g + 7)
            nc.sync.dma_start(wq_sb[:, gs, 0:4, :], wq_d[:, gs, 0:4, :])
        for g in range(4):
            gs = slice(7 * g, 7 * g + 7)
            nc.scalar.dma_start(wq_sb[:, gs, 4:NQ, :], wq_d[:, gs, 4:NQ, :])
        for g in range(4):
            gs = slice(7 * g, 7 * g + 7)
            nc.scalar.dma_start(x_sb[1][:, gs, :], xT_d[1, :, gs, :])
        nc.gpsimd.dma_start(bias_sb[:], bias_d[:])
        nc.gpsimd.dma_start(cos_sb[:], cos_d[:])
        nc.gpsimd.dma_start(sin_sb[:], sin_d[:])
        nc.gpsimd.dma_start(ident_sb[:], ident_d[:])
        nc.gpsimd.dma_start(ones_sb[:], ones_d[:])
        nc.gpsimd.dma_start(mask_sb[:], mask_d[:])
        warm = tpool.tile([128, 1], F32, tag="warm", name="warm")
        nc.scalar.activation(warm[:], bias_sb[:, 0:1],
                             mybir.ActivationFunctionType.Exp,
                             scale=1.0)

        def rope_drain(up, u, st, dst):
            # dst = rope(psum + bias): [x1 c - x2 s ; x2 c + x1 s]
            tmp = tpool.tile([128, TS], BF16, tag="tmp", name="tmp")
            nc.scalar.activation(tmp[:], up[:],
                                 mybir.ActivationFunctionType.Identity,
                                 bias=bias_sb[:, u:u + 1])
            c2 = cos_sb[:, st * TS:(st + 1) * TS]
            s2 = sin_sb[:, st * TS:(st + 1) * TS]
            scrA = spool.tile([128, TS], BF16, tag="scrA", name="scrA")
            scrB = spool.tile([128, TS], BF16, tag="scrB", name="scrB")
            nc.vector.tensor_mul(scrA[0:64, :], tmp[64:128, :], s2[64:128, :])
            nc.vector.tensor_mul(scrA[64:128, :], tmp[0:64, :], s2[0:64, :])
            nc.vector.tensor_mul(scrB[:], tmp[:], c2)
            nc.vector.tensor_add(dst, scrB[:], scrA[:])

        # ---------------- S1: per-stripe KV + Q, collective after st1 KV --
        # KV accumulators live on the den/av tags so the Q-pass "mm"
        # allocations have no dependency on the KV drains. V is exchanged
        # in H-major (vT) form and transposed by the XBAR on the
        # post-collective load, keeping the PE out of the exchange path.
        def kv_pass(st):
            kps = ps.tile([128, TS], F32, tag="den", bufs=2, name="kps")
            vps = ps.tile([128, TS], F32, tag="av", bufs=2, name="vps")
            for dc in range(DC):
                nc.tensor.matmul(kps[:], wkv_sb[:, dc, 0, :],
                                 x_sb[st][:, dc, :],
                                 start=(dc == 0), stop=(dc == DC - 1))
                nc.tensor.matmul(vps[:], wkv_sb[:, dc, 1, :],
                                 x_sb[st][:, dc, :],
                                 start=(dc == 0), stop=(dc == DC - 1))
            rope_drain(kps, NQ, st, blob[:, st * TS:(st + 1) * TS])
            nc.scalar.activation(blob[:, 1024 + st * TS:1024 + (st + 1) * TS],
                                 vps[:],
                                 mybir.ActivationFunctionType.Identity,
                                 bias=bias_sb[:, NQ + 1:NQ + 2])

        def q_pass(st, grp):
            ups = [ps.tile([128, TS], F32, tag="mm", bufs=4,
                           name=f"up{u}") for u in grp]
            for dc in range(DC):
                for i, u in enumerate(grp):
                    nc.tensor.matmul(ups[i][:], wq_sb[:, dc, u, :],
                                     x_sb[st][:, dc, :],
                                     start=(dc == 0), stop=(dc == DC - 1))
            for i, u in enumerate(grp):
                rope_drain(ups[i], u, st, qaT[u][:, st * TS:(st + 1) * TS])

        kv_pass(0)
        q_pass(0, range(0, 4))
        q_pass(0, range(4, NQ))
        kv_pass(1)

        # ---------------- pairwise K/V all-gather ------------------------
        # exchange triggers ride the scalar queue (weights are done by
        # now); the sync queue's long in-order wait chain would add
        # head-of-line latency here.
        nc.scalar.dma_start(in_b[:], blob[:])
        nc.gpsimd.collective_compute(
            "AllGather",
            mybir.AluOpType.bypass,
            replica_groups=[[0, 1], [2, 3], [4, 5], [6, 7]],
            ins=[in_b.opt()],
            outs=[out_b.opt()],
        )
        for r in (0, 1):
            nc.scalar.dma_start(kT_full[:, r::2, :], out_b[r, :, 0:8, :])
        for sc in range(NB):
            nc.scalar.dma_start_transpose(
                v_nat[:, sc, :], out_b[sc % 2, :, 8 + sc // 2, :])

        # Q stripe 1 overlaps the collective
        q_pass(1, range(0, 4))
        q_pass(1, range(4, NQ))

        # ---------------- S2 attention ----------------------------------
        for st in (0, 1):
            prof = W_ST[st]
            nsc = len(prof)
            cells = [(hq, sc) for hq in range(NQ) for sc in range(nsc)]
            pts = [None] * len(cells)

            def issue_scores(i, st=st, prof=prof, cells=cells, pts=pts):
                hq, sc = cells[i]
                o = (4 - prof[sc]) * 128
                sp = ps.tile([128, TS], F32, tag="mm", bufs=4, name="scps")
                nc.tensor.matmul(
                    sp[:, o:TS], kT_full[:, sc, :],
                    qaT[hq][:, st * TS + o:(st + 1) * TS],
                    start=True, stop=True)
                pt = ppool.tile([128, TS], BF16, tag="pt", name="pt")
                nc.scalar.activation(pt[:, o:TS], sp[:, o:TS],
                                     mybir.ActivationFunctionType.Exp,
                                     scale=SCALE)
                if sc >= 8 * st:
                    # multiplicative {0,1} mask on the SBUF P tile, on
                    # GpSimd: the DVE queue is saturated by the norm
                    # drains in stripe 0 and masks would chain behind them
                    pp = ((sc - 8 * st) // 2) * 128
                    nc.gpsimd.tensor_mul(pt[:, pp:pp + 128],
                                         pt[:, pp:pp + 128],
                                         mask_sb[:, sc, :])
                pts[i] = pt

            LA = 4
            den_ps = av_ps = None
            for i0 in range(min(LA, len(cells))):
                issue_scores(i0)
            for i, (hq, sc) in enumerate(cells):
                if i + LA < len(cells):
                    issue_scores(i + LA)
                o = (4 - prof[sc]) * 128
                if sc == 0:
                    den_ps = ps.tile([128, TS], F32, tag="den", bufs=2,
                                     name="den")
                    av_ps = ps.tile([128, TS], F32, tag="av", bufs=2,
                                    name="av")
                stf, spf = (sc == 0), (sc == nsc - 1)
                pt = pts[i]
                nc.tensor.matmul(den_ps[:, o:TS], ones_sb[:], pt[:, o:TS],
                                 start=stf, stop=spf)
                nc.tensor.matmul(av_ps[:, o:TS], v_nat[:, sc, :],
                                 pt[:, o:TS], start=stf, stop=spf)
                pts[i] = None
                if spf:
                    recip = rpool.tile([128, TS], F32, tag="recip",
                                       name="recip")
                    nc.vector.reciprocal_approx_fast(recip[:], den_ps[:])
                    nc.vector.tensor_mul(
                        qaT[hq][:, st * TS:(st + 1) * TS],
                        av_ps[:], recip[:])

        # ---------------- S3 output projection ---------------------------
        for nt in range(D // TS):
            wo_t = wpool.tile([128, NQ, TS], BF16, tag="wo", name=f"wo{nt}")
            nc.scalar.dma_start(wo_t[:], wo_d[:, :, nt * TS:(nt + 1) * TS])
            for tb0 in range(0, MYB, 2):
                yps = [ps.tile([128, TS], F32, tag="mm", bufs=4,
                               name=f"yp{j}") for j in range(2)]
                for u in range(NQ):
                    for j in range(2):
                        tbl = slice((tb0 + j) * 128, (tb0 + j + 1) * 128)
                        nc.tensor.matmul(yps[j][:], qaT[u][:, tbl],
                                         wo_t[:, u, :],
                                         start=(u == 0), stop=(u == NQ - 1))
                for j in range(2):
                    yout = ypool.tile([128, TS], BF16, tag="yo", name="yout")
                    if (tb0 + j + nt) % 2 == 0:
                        nc.scalar.copy(yout[:], yps[j][:])
                    else:
                        nc.vector.tensor_copy(yout[:], yps[j][:])
                    nc.sync.dma_start(
                        y_d[(tb0 + j) * 128:(tb0 + j + 1) * 128,
                            nt * TS:(nt + 1) * TS], yout[:])


def kernel(x, attn_mask, sin, cos, wq, wk, wv, wo, q_bias, k_bias, v_bias):
    x = np.asarray(x, np.float32)
    mask = np.asarray(attn_mask).astype(bool)
    sin = np.asarray(sin, np.float32)
    cos = np.asarray(cos, np.float32)
    wq = np.asarray(wq, np.float32)
    wk = np.asarray(wk, np.float32)
    wv = np.asarray(wv, np.float32)
    wo = np.asarray(wo, np.float32)
    q_bias = np.asarray(q_bias, np.float32).reshape(N, H)
    k_bias = np.asarray(k_bias, np.float32).reshape(KH, H)
    v_bias = np.asarray(v_bias, np.float32).reshape(KH, H)

    # causal-mask sanity: the kernel hardcodes the causal structure
    assert mask[0, 10, :11].all() and not mask[0, 10, 11:].any()

    BF = ml_dtypes.bfloat16
    xT = np.ascontiguousarray(x[0].T)                        # [D, T]
    c = cos[0].T                                             # [64, T]
    s = sin[0].T
    ident = np.eye(128, dtype=np.float32)
    ones128 = np.ones((128, 128), BF)

    # per-lane token index lists (even/odd 128-blocks)
    toks = {}
    for lane in (0, 1):
        toks[lane] = np.concatenate(
            [np.arange(b * 128, (b + 1) * 128) for b in range(lane, NB, 2)])

    # per-lane tensors
    xT16, cosT, sinT, maskC = {}, {}, {}, {}
    # multiplicative {0,1} masks applied to P after exp
    p_idx = np.arange(128)[:, None]
    j_idx = np.arange(128)[None, :]
    tri = np.where(p_idx > j_idx, 0.0, 1.0).astype(np.float32)
    full = np.zeros((128, 128), np.float32)
    zero = np.ones((128, 128), np.float32)
    for lane in (0, 1):
        tk = toks[lane]
        xl = xT[:, tk]                                       # [D, 1024]
        xT16[lane] = np.ascontiguousarray(
            xl.reshape(DC, 128, 2, TS).transpose(2, 1, 0, 3)).astype(BF)
        cc = np.concatenate([c[:, tk], c[:, tk]], 0)         # [128, 1024]
        ss = np.concatenate([s[:, tk], -s[:, tk]], 0)
        cosT[lane] = np.ascontiguousarray(cc).astype(BF)
        sinT[lane] = np.ascontiguousarray(ss).astype(BF)
        # mask content per key chunk sc (applied at block position
        # (sc - 8*st)//2 of the stripe): my block there is
        # b = sc + lane (sc even) or sc - 1 + lane (sc odd).
        mlist = []
        for sc in range(NB):
            if sc % 2 == lane:
                mlist.append(tri)          # diagonal block
            elif lane == 0:
                mlist.append(full)         # sc odd: b = sc-1 < sc
            else:
                mlist.append(zero)         # sc even: b = sc+1 > sc
        maskC[lane] = np.ascontiguousarray(
            np.stack(mlist, 1)).astype(BF)                   # [128, 16, 128]

    in_maps = []
    for cix in range(8):
        p = cix // 2
        lane = cix % 2
        qh = list(range(7 * p, 7 * p + 7))
        bcols = [q_bias[h] for h in qh] + [k_bias[p], v_bias[p]]
        wkvc = np.stack([wk[:, p, :], wv[:, p, :]], axis=1)  # [D, 2, 128]
        wkv16 = np.ascontiguousarray(
            wkvc.reshape(DC, 128, 2, 128).transpose(1, 0, 2, 3)).astype(BF)
        wqc = np.stack([wq[:, h, :] for h in qh], axis=1)    # [D, 7, 128]
        wq16 = np.ascontiguousarray(
            wqc.reshape(DC, 128, NQ, 128).transpose(1, 0, 2, 3)).astype(BF)
        biasT = np.stack(bcols, axis=1)                      # [128, 9]
        woT = np.ascontiguousarray(
            wo[qh].transpose(1, 0, 2)).astype(BF)            # [128, 7, D]
        in_maps.append({
            "xT16": xT16[lane], "wkv16": wkv16, "wq16": wq16,
            "biasT": biasT,
            "cosT": cosT[lane], "sinT": sinT[lane], "ident": ident,
            "maskC": maskC[lane], "woT": woT, "ones": ones128,
        })

    nc = build_program()
    res = run_bass_kernel_spmd(nc, in_maps, list(range(8)), trace=_TRACE)
    if _TRACE and res.exec_time_ns is not None:
        print(f"HW exec time: {res.exec_time_ns} ns")
    y = np.zeros((T, D), np.float64)
    for cix in range(8):
        lane = cix % 2
        r = res.results[cix]["y"].astype(np.float64)         # [1024, D]
        for i, b in enumerate(range(lane, NB, 2)):
            y[b * 128:(b + 1) * 128] += r[i * 128:(i + 1) * 128]
    return y.reshape(B, T, D).astype(np.float32)


# revision 23
# speedup vs baseline: 1.2266x; 1.2266x over previous
"""Trainium2 Bass kernel for GQA attention (B=1,T=2048,D=3584,N=28,KH=4,H=128).

v3 sharding: heads x sequence 2D split over 8 cores.
  Pair p = cores (2p, 2p+1) owns kv head p and query heads 7p..7p+6.
  Within a pair, lane l = core % 2 owns the even (l=0) / odd (l=1)
  128-token query blocks -- the even/odd interleave is the optimal
  balanced causal split (both lanes sum to 68 key-chunk visits, and the
  SPMD union profile ceil((16-sc)/2) adds only 4 phantom chunk-columns).

Per core:
  S1: 9 projection units (7 q heads + k + v) x 28 D-chunks over my 1024
      tokens. K is roped, V transposed to natural layout; both go into a
      pairwise DRAM AllGather so each core gets K/V for all 2048 keys
      while the PE streams the Q projections (collective fully hidden).
  S2: per (head, key chunk): scoresT = kT_chunk^T qT over the union
      suffix of my query blocks >= chunk, one data-driven mask add per
      chunk (tri/full/zero content from host), exp on ACT, den/av
      accumulated on alternating PSUM banks, DVE normalize -> attnT
      (aliased onto the qT tile).
  S3: y[my tokens, :] = sum_h attnT_h^T wo_h, wo streamed per 512-col
      tile, drains alternating ACT/DVE, host sums the 4 head-group
      partials per token row.
"""

import numpy as np
import ml_dtypes
from contextlib import ExitStack

import concourse.bass as bass
import concourse.bacc as bacc
import concourse.tile as tile
from concourse import mybir
from concourse.bass_utils import run_bass_kernel_spmd

F32 = mybir.dt.float32
BF16 = mybir.dt.bfloat16

B, T, D = 1, 2048, 3584
N, KH, H = 28, 4, 128
NQ = 7                   # query heads per core
NU = NQ + 2              # + k, v units
DC = D // 128            # 28 contraction chunks
TS = 512
NB = 16                  # 128-token blocks in T
MYB = 8                  # my blocks per core
SCALE = float(H) ** -0.5
MASKVAL = -30000.0

# union suffix profile: #active 128-col blocks at key chunk sc, per stripe
W_ST = {
    0: [4, 4, 3, 3, 2, 2, 1, 1],
    1: [4, 4, 4, 4, 4, 4, 4, 4, 4, 4, 3, 3, 2, 2, 1, 1],
}

_TRACE = False           # test.py flips this to get an NTFF profile


def build_program():
    nc = bacc.Bacc(None)
    _build_body(nc)
    nc.compile()
    return nc


def _build_body(nc):
    xT_d = nc.dram_tensor("xT16", [2, 128, DC, TS], BF16, kind="ExternalInput")
    wkv_d = nc.dram_tensor("wkv16", [128, DC, 2, 128], BF16,
                           kind="ExternalInput")
    wq_d = nc.dram_tensor("wq16", [128, DC, NQ, 128], BF16,
                          kind="ExternalInput")
    bias_d = nc.dram_tensor("biasT", [128, NU], F32, kind="ExternalInput")
    cos_d = nc.dram_tensor("cosT", [128, 1024], BF16, kind="ExternalInput")
    sin_d = nc.dram_tensor("sinT", [128, 1024], BF16, kind="ExternalInput")
    ident_d = nc.dram_tensor("ident", [128, 128], F32, kind="ExternalInput")
    mask_d = nc.dram_tensor("maskC", [128, NB, 128], BF16,
                            kind="ExternalInput")
    wo_d = nc.dram_tensor("woT", [128, NQ, D], BF16, kind="ExternalInput")
    ones_d = nc.dram_tensor("ones", [128, 128], BF16, kind="ExternalInput")
    y_d = nc.dram_tensor("y", [1024, D], BF16, kind="ExternalOutput")

    with tile.TileContext(nc) as tc, ExitStack() as ctx:
        persist = ctx.enter_context(tc.tile_pool(name="persist", bufs=1))

        wkv_sb = persist.tile([128, DC, 2, 128], BF16, tag="wkv")
        wq_sb = persist.tile([128, DC, NQ, 128], BF16, tag="wq")
        x_sb = [persist.tile([128, DC, TS], BF16, tag=f"x{st}",
                             name=f"x{st}")
                for st in (0, 1)]
        # qaT[u]: S1 writes roped qT here; S2 drains overwrite it with attnT
        # (last q read of a head-stripe precedes its attnT write).
        qaT = [persist.tile([128, 1024], BF16, tag=f"qaT{u}",
                            name=f"qaT{u}")
               for u in range(NQ)]
        blob = persist.tile([128, 2048], BF16, tag="blob")   # kT|v_nat mine
        # lane-major: [:, r, i, :] = lane r's block i (global chunk 2i+r).
        # Keeps the post-collective loads contiguous; the sc -> (sc%2,
        # sc//2) remap is compile-time and lane-independent.
        kT_full = persist.tile([128, 2, 8, 128], BF16, tag="ktf")
        v_nat = persist.tile([128, 2, 8, 128], BF16, tag="vnat")
        cos_sb = persist.tile([128, 1024], BF16, tag="cos")
        sin_sb = persist.tile([128, 1024], BF16, tag="sin")
        bias_sb = persist.tile([128, NU], F32, tag="bias")
        ident_sb = persist.tile([128, 128], F32, tag="ident")
        ones_sb = persist.tile([128, 128], BF16, tag="ones")
        mask_sb = persist.tile([128, NB, 128], BF16, tag="mask")

        tpool = ctx.enter_context(tc.tile_pool(name="tp", bufs=3))
        spool = ctx.enter_context(tc.tile_pool(name="sp", bufs=2))
        ppool = ctx.enter_context(tc.tile_pool(name="pp", bufs=5))
        rpool = ctx.enter_context(tc.tile_pool(name="rp", bufs=2))
        wpool = ctx.enter_context(tc.tile_pool(name="wp", bufs=3))
        ypool = ctx.enter_context(tc.tile_pool(name="yp", bufs=4))
        ps = ctx.enter_context(tc.tile_pool(name="ps", bufs=1, space="PSUM"))
        dram = ctx.enter_context(tc.tile_pool(name="dram", bufs=1,
                                              space="DRAM"))

        in_b = dram.tile([128, 2048], BF16, tag="inb")
        out_b = dram.tile([2, 128, NB, 128], BF16, tag="outb")

        # ---------------- loads (three queues, consumption order) --------
        # sync queue carries the PE-critical stream in exact consumption
        # order (KV st0 -> Q st0 grp1); scalar queue brings the later
        # wq halves + x stripe 1; gpsimd queue the small tables.
        nc.sync.dma_start(wkv_sb[:, 0:14, :, :], wkv_d[:, 0:14, :, :])
        nc.sync.dma_start(x_sb[0][:, 0:7, :], xT_d[0, :, 0:7, :])
        nc.sync.dma_start(wkv_sb[:, 14:28, :, :], wkv_d[:, 14:28, :, :])
        for g in range(1, 4):
            gs = slice(7 * g, 7 * g + 7)
            nc.sync.dma_start(x_sb[0][:, gs, :], xT_d[0, :, gs, :])
        for g in range(4):
            gs = slice(7 * g, 7 * g + 7)
            nc.sync.dma_start(wq_sb[:, gs, 0:4, :], wq_d[:, gs, 0:4, :])
        for g in range(4):
            gs = slice(7 * g, 7 * g + 7)
            nc.scalar.dma_start(wq_sb[:, gs, 4:NQ, :], wq_d[:, gs, 4:NQ, :])
        for g in range(4):
            gs = slice(7 * g, 7 * g + 7)
            nc.scalar.dma_start(x_sb[1][:, gs, :], xT_d[1, :, gs, :])
        nc.gpsimd.dma_start(bias_sb[:], bias_d[:])
        nc.gpsimd.dma_start(cos_sb[:], cos_d[:])
        nc.gpsimd.dma_start(sin_sb[:], sin_d[:])
        nc.gpsimd.dma_start(ident_sb[:], ident_d[:])
        nc.gpsimd.dma_start(ones_sb[:], ones_d[:])
        nc.gpsimd.dma_start(mask_sb[:], mask_d[:])
        warm = tpool.tile([128, 1], F32, tag="warm", name="warm")
        nc.scalar.activation(warm[:], bias_sb[:, 0:1],
                             mybir.ActivationFunctionType.Exp,
                             scale=1.0)

        def rope_drain(up, u, st, dst):
            # dst = rope(psum + bias): [x1 c - x2 s ; x2 c + x1 s]
            tmp = tpool.tile([128, TS], BF16, tag="tmp", name="tmp")
            nc.scalar.activation(tmp[:], up[:],
                                 mybir.ActivationFunctionType.Identity,
                                 bias=bias_sb[:, u:u + 1])
            c2 = cos_sb[:, st * TS:(st + 1) * TS]
            s2 = sin_sb[:, st * TS:(st + 1) * TS]
            scrA = spool.tile([128, TS], BF16, tag="scrA", name="scrA")
            scrB = spool.tile([128, TS], BF16, tag="scrB", name="scrB")
            nc.vector.tensor_mul(scrA[0:64, :], tmp[64:128, :], s2[64:128, :])
            nc.vector.tensor_mul(scrA[64:128, :], tmp[0:64, :], s2[0:64, :])
            nc.vector.tensor_mul(scrB[:], tmp[:], c2)
            nc.vector.tensor_add(dst, scrB[:], scrA[:])

        # ---------------- S1: per-stripe KV + Q, collective after st1 KV --
        # KV accumulators live on the den/av tags so the Q-pass "mm"
        # allocations have no dependency on the KV drains. V is exchanged
        # in H-major (vT) form and transposed by the XBAR on the
        # post-collective load, keeping the PE out of the exchange path.
        vtmps = [None, None]

        def kv_pass(st):
            kps = ps.tile([128, TS], F32, tag="den", bufs=2, name="kps")
            vps = ps.tile([128, TS], F32, tag="av", bufs=2, name="vps")
            for dc in range(DC):
                nc.tensor.matmul(kps[:], wkv_sb[:, dc, 0, :],
                                 x_sb[st][:, dc, :],
                                 start=(dc == 0), stop=(dc == DC - 1))
                nc.tensor.matmul(vps[:], wkv_sb[:, dc, 1, :],
                                 x_sb[st][:, dc, :],
                                 start=(dc == 0), stop=(dc == DC - 1))
            rope_drain(kps, NQ, st, blob[:, st * TS:(st + 1) * TS])
            vtmp = tpool.tile([128, TS], F32, tag="vtmp", bufs=2, name="vtmp")
            nc.scalar.activation(vtmp[:], vps[:],
                                 mybir.ActivationFunctionType.Identity,
                                 bias=bias_sb[:, NQ + 1:NQ + 2])
            vtmps[st] = vtmp

        def v_transposes(st):
            for j in range(4):
                vp = ps.tile([128, TS], F32, tag=("av" if j % 2 == 0
                                                  else "den"), bufs=2,
                             name="vp")
                nc.tensor.transpose(vp[:, 0:128],
                                    vtmps[st][:, j * 128:(j + 1) * 128],
                                    ident_sb[:])
                cl = 1024 + (4 * st + j) * 128
                nc.scalar.copy(blob[:, cl:cl + 128], vp[:, 0:128])

        def q_pass(st, grp):
            ups = [ps.tile([128, TS], F32, tag="mm", bufs=4,
                           name=f"up{u}") for u in grp]
            for dc in range(DC):
                for i, u in enumerate(grp):
                    nc.tensor.matmul(ups[i][:], wq_sb[:, dc, u, :],
                                     x_sb[st][:, dc, :],
                                     start=(dc == 0), stop=(dc == DC - 1))
            for i, u in enumerate(grp):
                rope_drain(ups[i], u, st, qaT[u][:, st * TS:(st + 1) * TS])

        kv_pass(0)
        q_pass(0, range(0, 4))
        v_transposes(0)
        q_pass(0, range(4, NQ))
        kv_pass(1)
        v_transposes(1)

        # ---------------- pairwise K/V all-gather ------------------------
        # exchange triggers ride the scalar queue (weights are done by
        # now); the sync queue's long in-order wait chain would add
        # head-of-line latency here.
        nc.scalar.dma_start(in_b[:], blob[:])
        nc.gpsimd.collective_compute(
            "AllGather",
            mybir.AluOpType.bypass,
            replica_groups=[[0, 1], [2, 3], [4, 5], [6, 7]],
            ins=[in_b.opt()],
            outs=[out_b.opt()],
        )
        for r in (0, 1):
            nc.scalar.dma_start(kT_full[:, r, :, :], out_b[r, :, 0:8, :])
            nc.scalar.dma_start(v_nat[:, r, :, :], out_b[r, :, 8:16, :])

        # Q stripe 1 overlaps the collective
        q_pass(1, range(0, 4))
        q_pass(1, range(4, NQ))

        # ---------------- S2 attention ----------------------------------
        for st in (0, 1):
            prof = W_ST[st]
            nsc = len(prof)
            cells = [(hq, sc) for hq in range(NQ) for sc in range(nsc)]
            pts = [None] * len(cells)

            def issue_scores(i, st=st, prof=prof, cells=cells, pts=pts):
                hq, sc = cells[i]
                o = (4 - prof[sc]) * 128
                sp = ps.tile([128, TS], F32, tag="mm", bufs=4, name="scps")
                nc.tensor.matmul(
                    sp[:, o:TS], kT_full[:, sc % 2, sc // 2, :],
                    qaT[hq][:, st * TS + o:(st + 1) * TS],
                    start=True, stop=True)
                pt = ppool.tile([128, TS], BF16, tag="pt", name="pt")
                nc.scalar.activation(pt[:, o:TS], sp[:, o:TS],
                                     mybir.ActivationFunctionType.Exp,
                                     scale=SCALE)
                if sc >= 8 * st:
                    # multiplicative {0,1} mask on the SBUF P tile, on
                    # GpSimd: the DVE queue is saturated by the norm
                    # drains in stripe 0 and masks would chain behind them
                    pp = ((sc - 8 * st) // 2) * 128
                    nc.gpsimd.tensor_mul(pt[:, pp:pp + 128],
                                         pt[:, pp:pp + 128],
                                         mask_sb[:, sc, :])
                pts[i] = pt

            LA = 4
            den_ps = av_ps = None
            for i0 in range(min(LA, len(cells))):
                issue_scores(i0)
            for i, (hq, sc) in enumerate(cells):
                if i + LA < len(cells):
                    issue_scores(i + LA)
                o = (4 - prof[sc]) * 128
                if sc == 0:
                    den_ps = ps.tile([128, TS], F32, tag="den", bufs=2,
                                     name="den")
                    av_ps = ps.tile([128, TS], F32, tag="av", bufs=2,
                                    name="av")
                stf, spf = (sc == 0), (sc == nsc - 1)
                pt = pts[i]
                nc.tensor.matmul(den_ps[:, o:TS], ones_sb[:], pt[:, o:TS],
                                 start=stf, stop=spf)
                nc.tensor.matmul(av_ps[:, o:TS], v_nat[:, sc % 2, sc // 2, :],
                                 pt[:, o:TS], start=stf, stop=spf)
                pts[i] = None
                if spf:
                    recip = rpool.tile([128, TS], F32, tag="recip",
                                       name="recip")
                    nc.vector.reciprocal_approx_fast(recip[:], den_ps[:])
                    nc.vector.tensor_mul(
                        qaT[hq][:, st * TS:(st + 1) * TS],
                        av_ps[:], recip[:])

        # ---------------- S3 output projection ---------------------------
        for nt in range(D // TS):
            wo_t = wpool.tile([128, NQ, TS], BF16, tag="wo", name=f"wo{nt}")
            nc.scalar.dma_start(wo_t[:], wo_d[:, :, nt * TS:(nt + 1) * TS])
            for tb0 in range(0, MYB, 2):
                yps = [ps.tile([128, TS], F32, tag="mm", bufs=4,
                               name=f"yp{j}") for j in range(2)]
                for u in range(NQ):
                    for j in range(2):
                        tbl = slice((tb0 + j) * 128, (tb0 + j + 1) * 128)
                        nc.tensor.matmul(yps[j][:], qaT[u][:, tbl],
                                         wo_t[:, u, :],
                                         start=(u == 0), stop=(u == NQ - 1))
                for j in range(2):
                    yout = ypool.tile([128, TS], BF16, tag="yo", name="yout")
                    if (tb0 + j + nt) % 2 == 0:
                        nc.scalar.copy(yout[:], yps[j][:])
                    else:
                        nc.vector.tensor_copy(yout[:], yps[j][:])
                    nc.sync.dma_start(
                        y_d[(tb0 + j) * 128:(tb0 + j + 1) * 128,
                            nt * TS:(nt + 1) * TS], yout[:])


def kernel(x, attn_mask, sin, cos, wq, wk, wv, wo, q_bias, k_bias, v_bias):
    x = np.asarray(x, np.float32)
    mask = np.asarray(attn_mask).astype(bool)
    sin = np.asarray(sin, np.float32)
    cos = np.asarray(cos, np.float32)
    wq = np.asarray(wq, np.float32)
    wk = np.asarray(wk, np.float32)
    wv = np.asarray(wv, np.float32)
    wo = np.asarray(wo, np.float32)
    q_bias = np.asarray(q_bias, np.float32).reshape(N, H)
    k_bias = np.asarray(k_bias, np.float32).reshape(KH, H)
    v_bias = np.asarray(v_bias, np.float32).reshape(KH, H)

    # causal-mask sanity: the kernel hardcodes the causal structure
    assert mask[0, 10, :11].all() and not mask[0, 10, 11:].any()

    BF = ml_dtypes.bfloat16
    xT = np.ascontiguousarray(x[0].T)                        # [D, T]
    c = cos[0].T                                             # [64, T]
    s = sin[0].T
    ident = np.eye(128, dtype=np.float32)
    ones128 = np.ones((128, 128), BF)

    # per-lane token index lists (even/odd 128-blocks)
    toks = {}
    for lane in (0, 1):
        toks[lane] = np.concatenate(
            [np.arange(b * 128, (b + 1) * 128) for b in range(lane, NB, 2)])

    # per-lane tensors
    xT16, cosT, sinT, maskC = {}, {}, {}, {}
    # multiplicative {0,1} masks applied to P after exp
    p_idx = np.arange(128)[:, None]
    j_idx = np.arange(128)[None, :]
    tri = np.where(p_idx > j_idx, 0.0, 1.0).astype(np.float32)
    full = np.zeros((128, 128), np.float32)
    zero = np.ones((128, 128), np.float32)
    for lane in (0, 1):
        tk = toks[lane]
        xl = xT[:, tk]                                       # [D, 1024]
        xT16[lane] = np.ascontiguousarray(
            xl.reshape(DC, 128, 2, TS).transpose(2, 1, 0, 3)).astype(BF)
        cc = np.concatenate([c[:, tk], c[:, tk]], 0)         # [128, 1024]
        ss = np.concatenate([s[:, tk], -s[:, tk]], 0)
        cosT[lane] = np.ascontiguousarray(cc).astype(BF)
        sinT[lane] = np.ascontiguousarray(ss).astype(BF)
        # mask content per key chunk sc (applied at block position
        # (sc - 8*st)//2 of the stripe): my block there is
        # b = sc + lane (sc even) or sc - 1 + lane (sc odd).
        mlist = []
        for sc in range(NB):
            if sc % 2 == lane:
                mlist.append(tri)          # diagonal block
            elif lane == 0:
                mlist.append(full)         # sc odd: b = sc-1 < sc
            else:
                mlist.append(zero)         # sc even: b = sc+1 > sc
        maskC[lane] = np.ascontiguousarray(
            np.stack(mlist, 1)).astype(BF)                   # [128, 16, 128]

    in_maps = []
    for cix in range(8):
        p = cix // 2
        lane = cix % 2
        qh = list(range(7 * p, 7 * p + 7))
        bcols = [q_bias[h] for h in qh] + [k_bias[p], v_bias[p]]
        wkvc = np.stack([wk[:, p, :], wv[:, p, :]], axis=1)  # [D, 2, 128]
        wkv16 = np.ascontiguousarray(
            wkvc.reshape(DC, 128, 2, 128).transpose(1, 0, 2, 3)).astype(BF)
        wqc = np.stack([wq[:, h, :] for h in qh], axis=1)    # [D, 7, 128]
        wq16 = np.ascontiguousarray(
            wqc.reshape(DC, 128, NQ, 128).transpose(1, 0, 2, 3)).astype(BF)
        biasT = np.stack(bcols, axis=1)                      # [128, 9]
        woT = np.ascontiguousarray(
            wo[qh].transpose(1, 0, 2)).astype(BF)            # [128, 7, D]
        in_maps.append({
            "xT16": xT16[lane], "wkv16": wkv16, "wq16": wq16,
            "biasT": biasT,
            "cosT": cosT[lane], "sinT": sinT[lane], "ident": ident,
            "maskC": maskC[lane], "woT": woT, "ones": ones128,
        })

    nc = build_program()
    res = run_bass_kernel_spmd(nc, in_maps, list(range(8)), trace=_TRACE)
    if _TRACE and res.exec_time_ns is not None:
        print(f"HW exec time: {res.exec_time_ns} ns")
    y = np.zeros((T, D), np.float64)
    for cix in range(8):
        lane = cix % 2
        r = res.results[cix]["y"].astype(np.float64)         # [1024, D]
        for i, b in enumerate(range(lane, NB, 2)):
            y[b * 128:(b + 1) * 128] += r[i * 128:(i + 1) * 128]
    return y.reshape(B, T, D).astype(np.float32)


# revision 25
# speedup vs baseline: 1.2342x; 1.0062x over previous
"""Trainium2 Bass kernel for GQA attention (B=1,T=2048,D=3584,N=28,KH=4,H=128).

v3 sharding: heads x sequence 2D split over 8 cores.
  Pair p = cores (2p, 2p+1) owns kv head p and query heads 7p..7p+6.
  Within a pair, lane l = core % 2 owns the even (l=0) / odd (l=1)
  128-token query blocks -- the even/odd interleave is the optimal
  balanced causal split (both lanes sum to 68 key-chunk visits, and the
  SPMD union profile ceil((16-sc)/2) adds only 4 phantom chunk-columns).

Per core:
  S1: 9 projection units (7 q heads + k + v) x 28 D-chunks over my 1024
      tokens. K is roped, V transposed to natural layout; both go into a
      pairwise DRAM AllGather so each core gets K/V for all 2048 keys
      while the PE streams the Q projections (collective fully hidden).
  S2: per (head, key chunk): scoresT = kT_chunk^T qT over the union
      suffix of my query blocks >= chunk, one data-driven mask add per
      chunk (tri/full/zero content from host), exp on ACT, den/av
      accumulated on alternating PSUM banks, DVE normalize -> attnT
      (aliased onto the qT tile).
  S3: y[my tokens, :] = sum_h attnT_h^T wo_h, wo streamed per 512-col
      tile, drains alternating ACT/DVE, host sums the 4 head-group
      partials per token row.
"""

import numpy as np
import ml_dtypes
from contextlib import ExitStack

import concourse.bass as bass
import concourse.bacc as bacc
import concourse.tile as tile
from concourse import mybir
from concourse.bass_utils import run_bass_kernel_spmd

F32 = mybir.dt.float32
BF16 = mybir.dt.bfloat16

B, T, D = 1, 2048, 3584
N, KH, H = 28, 4, 128
NQ = 7                   # query heads per core
NU = NQ + 2              # + k, v units
DC = D // 128            # 28 contraction chunks
TS = 512
NB = 16                  # 128-token blocks in T
MYB = 8                  # my blocks per core
SCALE = float(H) ** -0.5
MASKVAL = -30000.0

# union suffix profile: #active 128-col blocks at key chunk sc, per stripe
W_ST = {
    0: [4, 4, 3, 3, 2, 2, 1, 1],
    1: [4, 4, 4, 4, 4, 4, 4, 4, 4, 4, 3, 3, 2, 2, 1, 1],
}

_TRACE = False           # test.py flips this to get an NTFF profile


def build_program():
    nc = bacc.Bacc(None)
    _build_body(nc)
    nc.compile()
    return nc


def _build_body(nc):
    xT_d = nc.dram_tensor("xT16", [2, 128, DC, TS], BF16, kind="ExternalInput")
    wkv_d = nc.dram_tensor("wkv16", [128, DC, 2, 128], BF16,
                           kind="ExternalInput")
    wq_d = nc.dram_tensor("wq16", [128, DC, NQ, 128], BF16,
                          kind="ExternalInput")
    bias_d = nc.dram_tensor("biasT", [128, NU], F32, kind="ExternalInput")
    cos_d = nc.dram_tensor("cosT", [128, 1024], BF16, kind="ExternalInput")
    sin_d = nc.dram_tensor("sinT", [128, 1024], BF16, kind="ExternalInput")
    ident_d = nc.dram_tensor("ident", [128, 128], F32, kind="ExternalInput")
    mask_d = nc.dram_tensor("maskC", [128, NB, 128], BF16,
                            kind="ExternalInput")
    wo_d = nc.dram_tensor("woT", [128, NQ, D], BF16, kind="ExternalInput")
    ones_d = nc.dram_tensor("ones", [128, 128], BF16, kind="ExternalInput")
    y_d = nc.dram_tensor("y", [1024, D], BF16, kind="ExternalOutput")

    with tile.TileContext(nc) as tc, ExitStack() as ctx:
        persist = ctx.enter_context(tc.tile_pool(name="persist", bufs=1))

        wkv_sb = persist.tile([128, DC, 2, 128], BF16, tag="wkv")
        wq_sb = persist.tile([128, DC, NQ, 128], BF16, tag="wq")
        x_sb = [persist.tile([128, DC, TS], BF16, tag=f"x{st}",
                             name=f"x{st}")
                for st in (0, 1)]
        # qaT[u]: S1 writes roped qT here; S2 drains overwrite it with attnT
        # (last q read of a head-stripe precedes its attnT write).
        qaT = [persist.tile([128, 1024], BF16, tag=f"qaT{u}",
                            name=f"qaT{u}")
               for u in range(NQ)]
        blob = persist.tile([128, 2048], BF16, tag="blob")   # kT|v_nat mine
        # lane-major: [:, r, i, :] = lane r's block i (global chunk 2i+r).
        # Keeps the post-collective loads contiguous; the sc -> (sc%2,
        # sc//2) remap is compile-time and lane-independent.
        kT_full = persist.tile([128, 2, 8, 128], BF16, tag="ktf")
        v_nat = persist.tile([128, 2, 8, 128], BF16, tag="vnat")
        cos_sb = persist.tile([128, 1024], BF16, tag="cos")
        sin_sb = persist.tile([128, 1024], BF16, tag="sin")
        bias_sb = persist.tile([128, NU], F32, tag="bias")
        ident_sb = persist.tile([128, 128], F32, tag="ident")
        ones_sb = persist.tile([128, 128], BF16, tag="ones")
        mask_sb = persist.tile([128, NB, 128], BF16, tag="mask")

        tpool = ctx.enter_context(tc.tile_pool(name="tp", bufs=3))
        spool = ctx.enter_context(tc.tile_pool(name="sp", bufs=2))
        ppool = ctx.enter_context(tc.tile_pool(name="pp", bufs=5))
        rpool = ctx.enter_context(tc.tile_pool(name="rp", bufs=2))
        wpool = ctx.enter_context(tc.tile_pool(name="wp", bufs=3))
        ypool = ctx.enter_context(tc.tile_pool(name="yp", bufs=4))
        ps = ctx.enter_context(tc.tile_pool(name="ps", bufs=1, space="PSUM"))
        dram = ctx.enter_context(tc.tile_pool(name="dram", bufs=1,
                                              space="DRAM"))

        in_b = dram.tile([128, 2048], BF16, tag="inb")
        out_b = dram.tile([2, 128, NB, 128], BF16, tag="outb")

        # ---------------- loads (consumption order on the sync queue) ----
        # Phase order is Q st0 -> KV st0 -> KV st1 -> Q st1: the Q passes
        # consume wq+x slowly (~150GB/s), so they go first while the DMA
        # stream warms; the KV passes (fast consumers) then run on fully
        # resident x. One queue, exact consumption order.
        for g in range(4):
            gs = slice(7 * g, 7 * g + 7)
            nc.sync.dma_start(wq_sb[:, gs, 0:4, :], wq_d[:, gs, 0:4, :])
            nc.sync.dma_start(x_sb[0][:, gs, :], xT_d[0, :, gs, :])
        for g in range(4):
            gs = slice(7 * g, 7 * g + 7)
            nc.sync.dma_start(wq_sb[:, gs, 4:NQ, :], wq_d[:, gs, 4:NQ, :])
        for h in (0, 1):
            hs = slice(14 * h, 14 * h + 14)
            nc.sync.dma_start(wkv_sb[:, hs, :, :], wkv_d[:, hs, :, :])
        for g in range(4):
            gs = slice(7 * g, 7 * g + 7)
            nc.sync.dma_start(x_sb[1][:, gs, :], xT_d[1, :, gs, :])
        nc.gpsimd.dma_start(bias_sb[:], bias_d[:])
        nc.gpsimd.dma_start(cos_sb[:], cos_d[:])
        nc.gpsimd.dma_start(sin_sb[:], sin_d[:])
        nc.gpsimd.dma_start(ident_sb[:], ident_d[:])
        nc.gpsimd.dma_start(ones_sb[:], ones_d[:])
        nc.gpsimd.dma_start(mask_sb[:], mask_d[:])
        warm = tpool.tile([128, 1], F32, tag="warm", name="warm")
        nc.scalar.activation(warm[:], bias_sb[:, 0:1],
                             mybir.ActivationFunctionType.Exp,
                             scale=1.0)

        def rope_drain(up, u, st, dst):
            # dst = rope(psum + bias): [x1 c - x2 s ; x2 c + x1 s]
            tmp = tpool.tile([128, TS], BF16, tag="tmp", name="tmp")
            nc.scalar.activation(tmp[:], up[:],
                                 mybir.ActivationFunctionType.Identity,
                                 bias=bias_sb[:, u:u + 1])
            c2 = cos_sb[:, st * TS:(st + 1) * TS]
            s2 = sin_sb[:, st * TS:(st + 1) * TS]
            scrA = spool.tile([128, TS], BF16, tag="scrA", name="scrA")
            scrB = spool.tile([128, TS], BF16, tag="scrB", name="scrB")
            nc.vector.tensor_mul(scrA[0:64, :], tmp[64:128, :], s2[64:128, :])
            nc.vector.tensor_mul(scrA[64:128, :], tmp[0:64, :], s2[0:64, :])
            nc.vector.tensor_mul(scrB[:], tmp[:], c2)
            nc.vector.tensor_add(dst, scrB[:], scrA[:])

        # ---------------- S1: per-stripe KV + Q, collective after st1 KV --
        # KV accumulators live on the den/av tags so the Q-pass "mm"
        # allocations have no dependency on the KV drains. V is exchanged
        # in H-major (vT) form and transposed by the XBAR on the
        # post-collective load, keeping the PE out of the exchange path.
        vtmps = [None, None]

        def kv_pass(st):
            kps = ps.tile([128, TS], F32, tag="den", bufs=2, name="kps")
            vps = ps.tile([128, TS], F32, tag="av", bufs=2, name="vps")
            for dc in range(DC):
                nc.tensor.matmul(kps[:], wkv_sb[:, dc, 0, :],
                                 x_sb[st][:, dc, :],
                                 start=(dc == 0), stop=(dc == DC - 1))
                nc.tensor.matmul(vps[:], wkv_sb[:, dc, 1, :],
                                 x_sb[st][:, dc, :],
                                 start=(dc == 0), stop=(dc == DC - 1))
            rope_drain(kps, NQ, st, blob[:, st * TS:(st + 1) * TS])
            vtmp = tpool.tile([128, TS], F32, tag="vtmp", bufs=2, name="vtmp")
            nc.scalar.activation(vtmp[:], vps[:],
                                 mybir.ActivationFunctionType.Identity,
                                 bias=bias_sb[:, NQ + 1:NQ + 2])
            vtmps[st] = vtmp

        def v_transposes(st):
            for j in range(4):
                vp = ps.tile([128, TS], F32, tag=("av" if j % 2 == 0
                                                  else "den"), bufs=2,
                             name="vp")
                nc.tensor.transpose(vp[:, 0:128],
                                    vtmps[st][:, j * 128:(j + 1) * 128],
                                    ident_sb[:])
                cl = 1024 + (4 * st + j) * 128
                nc.scalar.copy(blob[:, cl:cl + 128], vp[:, 0:128])

        def q_pass(st, grp):
            ups = [ps.tile([128, TS], F32, tag="mm", bufs=4,
                           name=f"up{u}") for u in grp]
            for dc in range(DC):
                for i, u in enumerate(grp):
                    nc.tensor.matmul(ups[i][:], wq_sb[:, dc, u, :],
                                     x_sb[st][:, dc, :],
                                     start=(dc == 0), stop=(dc == DC - 1))
            for i, u in enumerate(grp):
                rope_drain(ups[i], u, st, qaT[u][:, st * TS:(st + 1) * TS])

        q_pass(0, range(0, 4))
        q_pass(0, range(4, NQ))
        kv_pass(0)
        v_transposes(0)
        kv_pass(1)
        v_transposes(1)

        # ---------------- pairwise K/V all-gather ------------------------
        # exchange triggers ride the scalar queue (weights are done by
        # now); the sync queue's long in-order wait chain would add
        # head-of-line latency here.
        nc.scalar.dma_start(in_b[:], blob[:])
        nc.gpsimd.collective_compute(
            "AllGather",
            mybir.AluOpType.bypass,
            replica_groups=[[0, 1], [2, 3], [4, 5], [6, 7]],
            ins=[in_b.opt()],
            outs=[out_b.opt()],
        )
        for r in (0, 1):
            nc.scalar.dma_start(kT_full[:, r, :, :], out_b[r, :, 0:8, :])
            nc.scalar.dma_start(v_nat[:, r, :, :], out_b[r, :, 8:16, :])

        # Q stripe 1 overlaps the collective
        q_pass(1, range(0, 4))
        q_pass(1, range(4, NQ))

        # ---------------- S2 attention ----------------------------------
        for st in (0, 1):
            prof = W_ST[st]
            nsc = len(prof)
            cells = [(hq, sc) for hq in range(NQ) for sc in range(nsc)]
            pts = [None] * len(cells)

            def issue_scores(i, st=st, prof=prof, cells=cells, pts=pts):
                hq, sc = cells[i]
                o = (4 - prof[sc]) * 128
                sp = ps.tile([128, TS], F32, tag="mm", bufs=4, name="scps")
                nc.tensor.matmul(
                    sp[:, o:TS], kT_full[:, sc % 2, sc // 2, :],
                    qaT[hq][:, st * TS + o:(st + 1) * TS],
                    start=True, stop=True)
                pt = ppool.tile([128, TS], BF16, tag="pt", name="pt")
                nc.scalar.activation(pt[:, o:TS], sp[:, o:TS],
                                     mybir.ActivationFunctionType.Exp,
                                     scale=SCALE)
                if sc >= 8 * st:
                    # multiplicative {0,1} mask on the SBUF P tile, on
                    # GpSimd: the DVE queue is saturated by the norm
                    # drains in stripe 0 and masks would chain behind them
                    pp = ((sc - 8 * st) // 2) * 128
                    nc.gpsimd.tensor_mul(pt[:, pp:pp + 128],
                                         pt[:, pp:pp + 128],
                                         mask_sb[:, sc, :])
                pts[i] = pt

            LA = 4
            den_ps = av_ps = None
            for i0 in range(min(LA, len(cells))):
                issue_scores(i0)
            for i, (hq, sc) in enumerate(cells):
                if i + LA < len(cells):
                    issue_scores(i + LA)
                o = (4 - prof[sc]) * 128
                if sc == 0:
                    den_ps = ps.tile([128, TS], F32, tag="den", bufs=2,
                                     name="den")
                    av_ps = ps.tile([128, TS], F32, tag="av", bufs=2,
                                    name="av")
                stf, spf = (sc == 0), (sc == nsc - 1)
                pt = pts[i]
                nc.tensor.matmul(den_ps[:, o:TS], ones_sb[:], pt[:, o:TS],
                                 start=stf, stop=spf)
                nc.tensor.matmul(av_ps[:, o:TS], v_nat[:, sc % 2, sc // 2, :],
                                 pt[:, o:TS], start=stf, stop=spf)
                pts[i] = None
                if spf:
                    recip = rpool.tile([128, TS], F32, tag="recip",
                                       name="recip")
                    nc.vector.reciprocal_approx_fast(recip[:], den_ps[:])
                    nc.vector.tensor_mul(
                        qaT[hq][:, st * TS:(st + 1) * TS],
                        av_ps[:], recip[:])

        # ---------------- S3 output projection ---------------------------
        for nt in range(D // TS):
            wo_t = wpool.tile([128, NQ, TS], BF16, tag="wo", name=f"wo{nt}")
            nc.scalar.dma_start(wo_t[:], wo_d[:, :, nt * TS:(nt + 1) * TS])
            for tb0 in range(0, MYB, 2):
                yps = [ps.tile([128, TS], F32, tag="mm", bufs=4,
                               name=f"yp{j}") for j in range(2)]
                for u in range(NQ):
                    for j in range(2):
                        tbl = slice((tb0 + j) * 128, (tb0 + j + 1) * 128)
                        nc.tensor.matmul(yps[j][:], qaT[u][:, tbl],
                                         wo_t[:, u, :],
                                         start=(u == 0), stop=(u == NQ - 1))
                for j in range(2):
                    yout = ypool.tile([128, TS], BF16, tag="yo", name="yout")
                    if (tb0 + j + nt) % 2 == 0:
                        nc.scalar.copy(yout[:], yps[j][:])
                    else:
                        nc.vector.tensor_copy(yout[:], yps[j][:])
                    nc.sync.dma_start(
                        y_d[(tb0 + j) * 128:(tb0 + j + 1) * 128,
                            nt * TS:(nt + 1) * TS], yout[:])


def kernel(x, attn_mask, sin, cos, wq, wk, wv, wo, q_bias, k_bias, v_bias):
    x = np.asarray(x, np.float32)
    mask = np.asarray(attn_mask).astype(bool)
    sin = np.asarray(sin, np.float32)
    cos = np.asarray(cos, np.float32)
    wq = np.asarray(wq, np.float32)
    wk = np.asarray(wk, np.float32)
    wv = np.asarray(wv, np.float32)
    wo = np.asarray(wo, np.float32)
    q_bias = np.asarray(q_bias, np.float32).reshape(N, H)
    k_bias = np.asarray(k_bias, np.float32).reshape(KH, H)
    v_bias = np.asarray(v_bias, np.float32).reshape(KH, H)

    # causal-mask sanity: the kernel hardcodes the causal structure
    assert mask[0, 10, :11].all() and not mask[0, 10, 11:].any()

    BF = ml_dtypes.bfloat16
    xT = np.ascontiguousarray(x[0].T)                        # [D, T]
    c = cos[0].T                                             # [64, T]
    s = sin[0].T
    ident = np.eye(128, dtype=np.float32)
    ones128 = np.ones((128, 128), BF)

    # per-lane token index lists (even/odd 128-blocks)
    toks = {}
    for lane in (0, 1):
        toks[lane] = np.concatenate(
            [np.arange(b * 128, (b + 1) * 128) for b in range(lane, NB, 2)])

    # per-lane tensors
    xT16, cosT, sinT, maskC = {}, {}, {}, {}
    # multiplicative {0,1} masks applied to P after exp
    p_idx = np.arange(128)[:, None]
    j_idx = np.arange(128)[None, :]
    tri = np.where(p_idx > j_idx, 0.0, 1.0).astype(np.float32)
    full = np.zeros((128, 128), np.float32)
    zero = np.ones((128, 128), np.float32)
    for lane in (0, 1):
        tk = toks[lane]
        xl = xT[:, tk]                                       # [D, 1024]
        xT16[lane] = np.ascontiguousarray(
            xl.reshape(DC, 128, 2, TS).transpose(2, 1, 0, 3)).astype(BF)
        cc = np.concatenate([c[:, tk], c[:, tk]], 0)         # [128, 1024]
        ss = np.concatenate([s[:, tk], -s[:, tk]], 0)
        cosT[lane] = np.ascontiguousarray(cc).astype(BF)
        sinT[lane] = np.ascontiguousarray(ss).astype(BF)
        # mask content per key chunk sc (applied at block position
        # (sc - 8*st)//2 of the stripe): my block there is
        # b = sc + lane (sc even) or sc - 1 + lane (sc odd).
        mlist = []
        for sc in range(NB):
            if sc % 2 == lane:
                mlist.append(tri)          # diagonal block
            elif lane == 0:
                mlist.append(full)         # sc odd: b = sc-1 < sc
            else:
                mlist.append(zero)         # sc even: b = sc+1 > sc
        maskC[lane] = np.ascontiguousarray(
            np.stack(mlist, 1)).astype(BF)                   # [128, 16, 128]

    in_maps = []
    for cix in range(8):
        p = cix // 2
        lane = cix % 2
        qh = list(range(7 * p, 7 * p + 7))
        bcols = [q_bias[h] for h in qh] + [k_bias[p], v_bias[p]]
        wkvc = np.stack([wk[:, p, :], wv[:, p, :]], axis=1)  # [D, 2, 128]
        wkv16 = np.ascontiguousarray(
            wkvc.reshape(DC, 128, 2, 128).transpose(1, 0, 2, 3)).astype(BF)
        wqc = np.stack([wq[:, h, :] for h in qh], axis=1)    # [D, 7, 128]
        wq16 = np.ascontiguousarray(
            wqc.reshape(DC, 128, NQ, 128).transpose(1, 0, 2, 3)).astype(BF)
        biasT = np.stack(bcols, axis=1)                      # [128, 9]
        woT = np.ascontiguousarray(
            wo[qh].transpose(1, 0, 2)).astype(BF)            # [128, 7, D]
        in_maps.append({
            "xT16": xT16[lane], "wkv16": wkv16, "wq16": wq16,
            "biasT": biasT,
            "cosT": cosT[lane], "sinT": sinT[lane], "ident": ident,
            "maskC": maskC[lane], "woT": woT, "ones": ones128,
        })

    nc = build_program()
    res = run_bass_kernel_spmd(nc, in_maps, list(range(8)), trace=_TRACE)
    if _TRACE and res.exec_time_ns is not None:
        print(f"HW exec time: {res.exec_time_ns} ns")
    y = np.zeros((T, D), np.float64)
    for cix in range(8):
        lane = cix % 2
        r = res.results[cix]["y"].astype(np.float64)         # [1024, D]
        for i, b in enumerate(range(lane, NB, 2)):
            y[b * 128:(b + 1) * 128] += r[i * 128:(i + 1) * 128]
    return y.reshape(B, T, D).astype(np.float32)
